# revision 4
# baseline (speedup 1.0000x reference)
"""CrossInteraction kernel for TRN2, 8-core data parallel.

Math: interaction[b,i,j] = x1[b,i] * x2[b,j]
  mean_dim1[b,i] = x1[b,i] * mean_j(x2[b,j])
  mean_dim2[b,j] = x2[b,j] * mean_i(x1[b,i])
  out = concat([mean_dim1, mean_dim2], axis=1)   # (B, DIM1+DIM2)

So the (B, DIM1, DIM2) interaction tensor is never materialized: per batch
row we need one row-mean of x1, one row-mean of x2, and two scaled copies.

Sharding: pure data parallel over batch — 256 rows / 8 cores = 32 rows/core.
"""

import numpy as np

import concourse.bass as bass
import concourse.bacc as bacc
import concourse.tile as tile
from concourse import mybir
from concourse.bass_utils import run_bass_kernel_spmd

BATCH, DIM1, DIM2 = 256, 512, 1024
N_CORES = 8
B_LOC = BATCH // N_CORES  # 32 rows per core

_FP32 = mybir.dt.float32


def build_nc() -> bass.Bass:
    nc = bacc.Bacc(
        "TRN2", target_bir_lowering=False, debug=False, num_devices=N_CORES
    )
    x1 = nc.dram_tensor("x1", [B_LOC, DIM1], _FP32, kind="ExternalInput").ap()
    x2 = nc.dram_tensor("x2", [B_LOC, DIM2], _FP32, kind="ExternalInput").ap()
    out = nc.dram_tensor("out", [B_LOC, DIM1 + DIM2], _FP32, kind="ExternalOutput").ap()

    with tile.TileContext(nc) as tc:
        with tc.tile_pool(name="p", bufs=1) as pool:
            x1_t = pool.tile([B_LOC, DIM1], _FP32)
            x2_t = pool.tile([B_LOC, DIM2], _FP32)
            nc.gpsimd.dma_start(x1_t[:], x1[:])
            nc.gpsimd.dma_start(x2_t[:], x2[:])

            s1 = pool.tile([B_LOC, 1], _FP32)
            s2 = pool.tile([B_LOC, 1], _FP32)
            nc.vector.reduce_sum(s1[:], x1_t[:], axis=mybir.AxisListType.X)
            nc.vector.reduce_sum(s2[:], x2_t[:], axis=mybir.AxisListType.X)

            o = pool.tile([B_LOC, DIM1 + DIM2], _FP32)
            # out[:, :DIM1] = x1 * (sum(x2)/DIM2); out[:, DIM1:] = x2 * (sum(x1)/DIM1)
            nc.vector.tensor_scalar(
                o[:, :DIM1], x1_t[:], s2[:], 1.0 / DIM2,
                mybir.AluOpType.mult, mybir.AluOpType.mult,
            )
            nc.vector.tensor_scalar(
                o[:, DIM1:], x2_t[:], s1[:], 1.0 / DIM1,
                mybir.AluOpType.mult, mybir.AluOpType.mult,
            )
            nc.gpsimd.dma_start(out[:], o[:])
    nc.compile()
    return nc


def run(x1: np.ndarray, x2: np.ndarray, trace: bool = False):
    """Build + run on 8 cores; returns (full_output, BassKernelResults)."""
    nc = build_nc()
    x1 = np.ascontiguousarray(np.asarray(x1, dtype=np.float32))
    x2 = np.ascontiguousarray(np.asarray(x2, dtype=np.float32))
    in_maps = [
        {
            "x1": x1[i * B_LOC:(i + 1) * B_LOC],
            "x2": x2[i * B_LOC:(i + 1) * B_LOC],
        }
        for i in range(N_CORES)
    ]
    res = run_bass_kernel_spmd(nc, in_maps, list(range(N_CORES)), trace=trace)
    full = np.concatenate([r["out"] for r in res.results], axis=0)
    return full, res


def kernel(x1: np.ndarray, x2: np.ndarray) -> np.ndarray:
    full, _ = run(x1, x2, trace=False)
    return full
